# revision 72
# baseline (speedup 1.0000x reference)
"""BERT layer (B=8, S=1024, E=1024, F=4096) on 8 trn2 NeuronCores.

Strategy: pure data-parallel over batch (1 element per core, no collectives).
Per-core kernel keeps activations feature-major ([features, tokens]) so every
weight matmul uses the natural [in, out] weight block as the PE stationary
operand.

The attention path (QKV projections, scores, AV, Wd) runs on the PE in
fp8-e4m3 with DoubleRow perf mode (2x the bf16 rate, measured on HW);
accumulation stays fp32 in PSUM, so only operand-quantization noise is
added (1.73e-2 max rel err measured vs the 2e-2 gate). The FF block
(h1@Wi, ff@Wo) dominates the remaining flops but is too error-sensitive for
full fp8 (activation-quantization noise feeds the output LN undiluted), so
it stays fp16 except the first quarter of FF1's contraction range, which
runs fp8-DoubleRow. LayerNorm statistics run on the PE via ones-vector matmuls
interleaved into the producing loops; residuals and LN chains stay
fp16/fp32. Scale folding: x is staged as fp8(16*x), weights as fp8(32*W),
descale (1/512) is folded into the PSUM-evacuation activation; the LN1
residual+bias (x + bd + bv@Wd) is folded into the fp16 copy of x on the
host; exp runs as exp(scores/32 - 2) so fp8 attention weights stay inside
e4m3 range, the -2 cancelling in the softmax normalization.
"""

import sys

for _p in ("/opt/trn_rl_repo", "/root/.axon_site/_ro/trn_rl_repo"):
    if _p not in sys.path:
        sys.path.append(_p)

import numpy as np
import ml_dtypes

import concourse.bass as bass  # noqa: F401
import concourse.mybir as mybir
from concourse import bacc
from concourse.bass_utils import run_bass_kernel_spmd  # noqa: F401
from concourse.tile import TileContext

B, S, E, F = 8, 1024, 1024, 4096
P = 128
NE = E // P     # 8 tiles along E
NF = F // P     # 32 tiles along F
NS = S // P     # 8 tiles along S
C = 512         # free-dim chunk (one fp32 psum bank)
NC = S // C     # 2 chunks along S
EPS = 1e-12
AF = mybir.ActivationFunctionType
ALU = mybir.AluOpType
F32 = mybir.dt.float32
BF16 = mybir.dt.float16
FP8 = mybir.dt.float8e4
DR = mybir.MatmulPerfMode.DoubleRow
NPBF16 = mybir.dt.np(BF16)
NPFP8 = ml_dtypes.float8_e4m3

XS = 16.0        # fp8 scale on x
WS = 32.0        # fp8 scale on attention-path weights
DS = 1.0 / (XS * WS)   # descale after x8 @ w8 psum


def _ln_scalars(nc, SM, psum_pool, s1, s2, dim, epst, ones_row, pfx, w=C):
    """From column-sum psum APs s1=sum(z), s2=sum(z^2) [1,w] build two fp16
    SBUF broadcast tiles [P,w] (as [:, :w] views of full-width scratch):
    bcax = rstd, bcbx = -mu*rstd."""
    negmu = SM.tile([1, C], F32, tag="lnt0", name=f"negmu_{pfx}")
    musq = SM.tile([1, C], F32, tag="lnt1", name=f"musq_{pfx}")
    var = SM.tile([1, C], F32, tag="lnt2", name=f"var_{pfx}")
    std = SM.tile([1, C], F32, tag="lnt1", name=f"std_{pfx}")
    rstd = SM.tile([1, C], F32, tag="lnt2", name=f"rstd_{pfx}")
    nmr = SM.tile([1, C], BF16, tag="lnt3", name=f"nmr_{pfx}")
    rstd_r = SM.tile([1, C], BF16, tag="lnt4", name=f"rstdr_{pfx}")
    nc.vector.tensor_scalar_mul(negmu[:, :w], s1, -1.0 / dim)
    nc.vector.tensor_tensor(musq[:, :w], negmu[:, :w], negmu[:, :w], op=ALU.mult)
    nc.vector.scalar_tensor_tensor(
        var[:, :w], s2, 1.0 / dim, musq[:, :w], op0=ALU.mult, op1=ALU.subtract
    )
    nc.scalar.activation(std[:, :w], var[:, :w], AF.Sqrt, bias=epst[0:1, 0:1])
    nc.vector.reciprocal_approx_fast(out=rstd[:, :w], in_=std[:, :w])
    nc.vector.tensor_tensor(nmr[:, :w], negmu[:, :w], rstd[:, :w], op=ALU.mult)
    nc.vector.tensor_copy(rstd_r[:, :w], rstd[:, :w])
    bca = psum_pool.tile([P, C], F32, tag="bca", bufs=1, name=f"bca_{pfx}")
    bcb = psum_pool.tile([P, C], F32, tag="bcb", bufs=1, name=f"bcb_{pfx}")
    nc.tensor.matmul(bca[:, :w], ones_row[:], rstd_r[:, :w], start=True, stop=True)
    nc.tensor.matmul(bcb[:, :w], ones_row[:], nmr[:, :w], start=True, stop=True)
    bcax = SM.tile([P, C], BF16, tag="bcax", name=f"bcax_{pfx}")
    bcbx = SM.tile([P, C], BF16, tag="bcbx", name=f"bcbx_{pfx}")
    nc.vector.tensor_copy(bcax[:, :w], bca[:, :w])
    nc.vector.tensor_copy(bcbx[:, :w], bcb[:, :w])
    return bcax, bcbx


def _ln_normalize(nc, R3, z_tile, bcax, bcbx, g, b, n, dst_ap):
    """dst = ((z * rstd) + (-mu*rstd)) * g[n] + b[n] for one [P, C] tile."""
    t1 = R3.tile([P, C], BF16, tag="tmp")
    nc.vector.tensor_mul(t1[:], z_tile, bcax[:])
    nc.vector.tensor_add(t1[:], t1[:], bcbx[:])
    nc.scalar.activation(
        dst_ap, t1[:], AF.Identity, bias=b[:, n:n + 1], scale=g[:, n:n + 1]
    )


def build():
    nc = bacc.Bacc("TRN2", target_bir_lowering=False, debug=False)

    xT8_d = nc.dram_tensor("xT8", [E, S], FP8, kind="ExternalInput")
    xTc_d = nc.dram_tensor("xTc", [E, S], BF16, kind="ExternalInput")
    # all weight tiles stored pre-transposed so every staging DMA is a
    # contiguous per-partition copy (the strided k->p rearrange DMAs moved
    # to host numpy)
    wq_d = nc.dram_tensor("wq", [NE, P, NE, P], FP8, kind="ExternalInput")
    wk_d = nc.dram_tensor("wk", [NE, P, NE, P], FP8, kind="ExternalInput")
    wv_d = nc.dram_tensor(
        "wv", [NC, NE // 2, P, 2, C], FP8, kind="ExternalInput"
    )
    wd_d = nc.dram_tensor("wd", [NE, P, NE, P], FP8, kind="ExternalInput")
    # FF1 runs its first 2 k-tiles (256 of 1024 contraction rows) in fp8
    # DoubleRow and the rest in fp16 -- partial-K quantization keeps the
    # added LN2 noise inside the error budget while saving 1 of 8 chain
    # matmuls. The fp16 weight half carries the 512x scale of the fp8 path
    # (folded on host) so both accumulate into one psum chain.
    wi8_d = nc.dram_tensor("wi8", [NF, P, 2, P], FP8, kind="ExternalInput")
    wi_d = nc.dram_tensor("wi", [NF, P, NE - 2, P], BF16, kind="ExternalInput")
    wo_d = nc.dram_tensor("wo", [NE, 4, P, 8, P], BF16, kind="ExternalInput")
    # bias columns: [bq, bk, -, -, g1, b1, bo, g2, b2] -> [P, 9*NE]
    bias_d = nc.dram_tensor("biases", [P, 9 * NE], F32, kind="ExternalInput")
    bi_d = nc.dram_tensor("bi_cols", [P, NF], F32, kind="ExternalInput")
    ones_d = nc.dram_tensor("ones_in", [P, 1], BF16, kind="ExternalInput")
    ones8_d = nc.dram_tensor("ones8_in", [P, 1], FP8, kind="ExternalInput")
    onesrow_d = nc.dram_tensor("onesrow_in", [1, P], BF16, kind="ExternalInput")
    # fp16 output (host upconverts): strictly less work -- halves the
    # tail out-DMA and drain; correctness verified on HW (rel err
    # unchanged at 1.7327e-2)
    outT_d = nc.dram_tensor("outT", [E, S], BF16, kind="ExternalOutput")

    with TileContext(nc) as tc:
        with (
            tc.tile_pool(name="persist", bufs=1) as PP,
            tc.tile_pool(name="wstage", bufs=4) as WS_,
            tc.tile_pool(name="small", bufs=1) as SM,
            tc.tile_pool(name="rot3", bufs=6) as R3,
            tc.tile_pool(name="rot2", bufs=2) as R2,
        ):
            # ---- x staged fp8 (matmuls) + fp16 with LN1 bias folded ----
            xT8 = PP.tile([P, NE, S], FP8, tag="xT8")
            xTc = PP.tile([P, NE, S], BF16, tag="xTc")
            # stage kp=0's operands issue first (each dma_start costs
            # ~0.6us of Sync-engine issue time), then kp=1's
            wvst0 = WS_.tile([P, 2, C], FP8, tag="wvst", bufs=3, name="wvst00")
            wvst1 = WS_.tile([P, 2, C], FP8, tag="wvst", bufs=3, name="wvst01")
            nc.sync.dma_start(wvst0[:], wv_d[0, 0])
            # one batched 4-tile DMA (same 1KB-run descriptor pattern as 4
            # separate ones) saves 3 Sync issue slots on the critical path
            nc.sync.dma_start(
                xT8[:, 0:4, :],
                xT8_d[0:4 * P, :].rearrange("(k p) s -> p k s", k=4),
            )
            nc.sync.dma_start(wvst1[:], wv_d[0, 1])

            # ================= v = x @ Wv (token-major, no bias) ============
            v8 = PP.tile([P, NS, E], FP8, tag="v8")
            with tc.tile_pool(name="pv", bufs=1, space="PSUM") as PV:
                for c in range(NC):
                    pvs = [
                        PV.tile([P, C], F32, tag=f"pv{s_t}", name=f"pv{s_t}_{c}")
                        for s_t in range(NS)
                    ]
                    for kp in range(NE // 2):
                        if c == 0 and kp > 1:
                            for k in (2 * kp, 2 * kp + 1):
                                nc.sync.dma_start(
                                    xT8[:, k, :], xT8_d[k * P:(k + 1) * P, :]
                                )
                        if c == 0 and kp == 0:
                            wvst = wvst0
                        elif c == 0 and kp == 1:
                            wvst = wvst1
                        else:
                            wvst = WS_.tile([P, 2, C], FP8, tag="wvst", bufs=3)
                            nc.sync.dma_start(wvst[:], wv_d[c, kp])
                        for s_t in range(NS):
                            nc.tensor.matmul(
                                pvs[s_t][:],
                                xT8[:, 2 * kp:2 * kp + 2, s_t * P:(s_t + 1) * P],
                                wvst[:],
                                start=(kp == 0),
                                stop=(kp == NE // 2 - 1),
                                perf_mode=DR,
                            )
                    for s_t in range(NS):
                        # v8 = fp8(16 * v) = ps * (16 * DS); alternate the
                        # evacs between scalar and vector engines so the
                        # psum WAR chain for chunk c+1 clears 2x faster
                        if s_t % 2 == 0:
                            nc.scalar.activation(
                                v8[:, s_t, c * C:(c + 1) * C], pvs[s_t][:],
                                AF.Identity, scale=XS * DS,
                            )
                        else:
                            nc.vector.tensor_scalar_mul(
                                v8[:, s_t, c * C:(c + 1) * C], pvs[s_t][:],
                                XS * DS,
                            )

            # ---- constants ----
            ones = SM.tile([P, 1], BF16, tag="ones")
            nc.sync.dma_start(ones[:], ones_d[:])
            ones8 = SM.tile([P, 1], FP8, tag="ones8")
            nc.sync.dma_start(ones8[:], ones8_d[:])
            ones_row = SM.tile([1, P], BF16, tag="onesrow")
            nc.sync.dma_start(ones_row[:], onesrow_d[:])
            epst = SM.tile([1, 1], F32, tag="epst")
            nc.vector.memset(epst[:], EPS)
            neg2 = SM.tile([P, 1], F32, tag="neg2")
            nc.vector.memset(neg2[:], -2.0)
            biases = SM.tile([P, 9 * NE], F32, tag="biases")
            nc.sync.dma_start(biases[:], bias_d[:])
            bq = biases[:, 0 * NE:1 * NE]
            bk = biases[:, 1 * NE:2 * NE]
            g1 = biases[:, 4 * NE:5 * NE]
            b1 = biases[:, 5 * NE:6 * NE]
            bo = biases[:, 6 * NE:7 * NE]
            g2 = biases[:, 7 * NE:8 * NE]
            b2 = biases[:, 8 * NE:9 * NE]
            bicol = SM.tile([P, NF], F32, tag="bicol")
            nc.sync.dma_start(bicol[:], bi_d[:])

            # ================= qT / kT (fp8 out, bias added) ================
            qT8 = PP.tile([P, NE, S], FP8, tag="qT8")
            kT8 = PP.tile([P, NE, S], FP8, tag="kT8")
            with tc.tile_pool(name="pqk", bufs=3, space="PSUM") as PQK:
                for wi_, (w_d, dst, bias_ap) in enumerate(
                    ((wq_d, qT8, bq), (wk_d, kT8, bk))
                ):
                    for n in range(NE):
                        wst = WS_.tile([P, NE, P], FP8, tag="wst8")
                        nc.sync.dma_start(wst[:], w_d[n])
                        if wi_ == 0:
                            # spread the xTc residual prefetch (2 MB) across
                            # the qk weight stages so it never blocks a
                            # latency-critical staging DMA
                            nc.sync.dma_start(
                                xTc[:, n, :], xTc_d[n * P:(n + 1) * P, :]
                            )
                        for c in range(NC):
                            ps = PQK.tile([P, C], F32, tag="pqk", bufs=6)
                            for kp in range(NE // 2):
                                nc.tensor.matmul(
                                    ps[:],
                                    wst[:, 2 * kp:2 * kp + 2, :],
                                    xT8[:, 2 * kp:2 * kp + 2, c * C:(c + 1) * C],
                                    start=(kp == 0),
                                    stop=(kp == NE // 2 - 1),
                                    perf_mode=DR,
                                )
                            # alternate evac engines (scalar act / DVE STT)
                            if (2 * n + c) % 2 == 0:
                                nc.scalar.activation(
                                    dst[:, n, c * C:(c + 1) * C], ps[:],
                                    AF.Identity,
                                    bias=bias_ap[:, n:n + 1], scale=DS,
                                )
                            else:
                                nc.vector.scalar_tensor_tensor(
                                    dst[:, n, c * C:(c + 1) * C], ps[:], DS,
                                    bias_ap[:, n:n + 1].to_broadcast([P, C]),
                                    op0=ALU.mult, op1=ALU.add,
                                )

            # ================= attention ====================================
            z1 = PP.tile([P, NE, S], BF16, tag="z1")
            h1 = PP.tile([P, NE, S], BF16, tag="h1")
            h18 = PP.tile([P, 2, S], FP8, tag="h18")  # fp8(16*h1), k-tiles 0,1

            ATT_B = tc.tile_pool(name="attB", bufs=1, space="PSUM")
            PB = ATT_B.__enter__()
            ATT_A = tc.tile_pool(name="attA", bufs=1, space="PSUM")
            PA = ATT_A.__enter__()

            def attn_scores(qb):
                qs = slice(qb * C, (qb + 1) * C)
                wT8 = PP.tile([P, NS, C], FP8, tag="wT8", name=f"wT8_{qb}")
                for j in range(NS):
                    ps_sT = PA.tile(
                        [P, C], F32, tag="psc", bufs=3, name=f"ps_sT{qb}_{j}"
                    )
                    for kp in range(NE // 2):
                        nc.tensor.matmul(
                            ps_sT[:],
                            kT8[:, 2 * kp:2 * kp + 2, j * P:(j + 1) * P],
                            qT8[:, 2 * kp:2 * kp + 2, qs],
                            start=(kp == 0),
                            stop=(kp == NE // 2 - 1),
                            perf_mode=DR,
                        )
                    # wT = exp(scores/32 - 2); -2 cancels in normalization,
                    # keeps fp8 range (max exp(5.3-2) ~ 27 << 240)
                    nc.scalar.activation(
                        wT8[:, j, :], ps_sT[:], AF.Exp,
                        scale=1.0 / 32.0, bias=neg2[:, 0:1],
                    )
                return wT8

            def attn_den(qb, wT8):
                ps_den = PA.tile([1, C], F32, tag="pden", bufs=1, name=f"psden{qb}")
                for j in range(NS):
                    # dual-fp8 LDWEIGHTS rejects free-size-2 weights, so the
                    # denominator reduction stays non-DoubleRow (fp8 at 1 cyc)
                    nc.tensor.matmul(
                        ps_den[:], ones8[:], wT8[:, j, :],
                        start=(j == 0), stop=(j == NS - 1),
                    )
                rec = SM.tile([1, C], F32, tag="rcp", name=f"rec{qb}")
                nc.vector.reciprocal_approx_fast(out=rec[:], in_=ps_den[:])
                rec_r = SM.tile([1, C], BF16, tag="rcpr", name=f"recr{qb}")
                nc.vector.tensor_copy(rec_r[:], rec[:])
                return rec_r

            def attn_bcast(qb, rec_r):
                bcq = PA.tile([P, C], F32, tag="bcq", bufs=1, name=f"bcq{qb}")
                nc.tensor.matmul(
                    bcq[:], ones_row[:], rec_r[:], start=True, stop=True,
                )
                recb = SM.tile([P, C], F32, tag="rcb", name=f"recb{qb}")
                nc.vector.tensor_copy(recb[:], bcq[:])
                return recb

            def attn_av(qb, wT8, recb):
                # astg8 = fp8(16*attn) = fp8(ps_a * (1/den)); the 16 comes
                # from v8 already carrying 16*v
                astg8 = PP.tile(
                    [P, NE, C], FP8, tag="astg8", bufs=2, name=f"astg8_{qb}"
                )
                for e_t in range(NE):
                    ps_a = PB.tile(
                        [P, C], F32, tag="pav", bufs=3, name=f"ps_a{qb}_{e_t}"
                    )
                    for jp in range(NS // 2):
                        nc.tensor.matmul(
                            ps_a[:],
                            v8[:, 2 * jp:2 * jp + 2, e_t * P:(e_t + 1) * P],
                            wT8[:, 2 * jp:2 * jp + 2, :],
                            start=(jp == 0),
                            stop=(jp == NS // 2 - 1),
                            perf_mode=DR,
                        )
                    nc.vector.tensor_mul(astg8[:, e_t, :], ps_a[:], recb[:])
                return astg8

            # Interleave the two query blocks so the PE never waits on the
            # DVE reciprocal chains: block-1 scores fill block-0's rec
            # latency, and rec1 computes during bcast0 + av0.
            wT0 = attn_scores(0)
            rec_r0 = attn_den(0, wT0)
            wT1 = attn_scores(1)
            rec_r1 = attn_den(1, wT1)
            recb0 = attn_bcast(0, rec_r0)
            astg0 = attn_av(0, wT0, recb0)
            recb1 = attn_bcast(1, rec_r1)
            astg1 = attn_av(1, wT1, recb1)
            ATT_A.__exit__(None, None, None)
            ATT_B.__exit__(None, None, None)

            # ====== post-attention scope: Wd + LN1 + FF + LN2 ==============
            with tc.tile_pool(name="postpsum", bufs=1, space="PSUM") as PO:

                def sq_prewarm(pfx):
                    """Load the Sqrt table right after the last Square of a
                    stats loop, so the LN scalar chain's Sqrt needs no
                    ACT_TABLE_LOAD on the critical path."""
                    sqw = SM.tile([1, 1], F32, tag="sqwarm", name=f"sqw_{pfx}")
                    nc.scalar.activation(sqw[:], epst[:], AF.Sqrt)

                def wd_chunk(c, astg8, mid_cbs=()):
                    """Wd matmuls + z1 evac + inline LN1 stats for chunk c.
                    z1 = ps * (1/512) + (x + bd + bv@Wd)   [bias folded on
                    host into xTc; astg carries 16x, wd carries 32x]."""
                    cs = slice(c * C, (c + 1) * C)
                    s1 = PO.tile([1, C], F32, tag="pstat1", bufs=1, name=f"l1s1_{c}")
                    s2 = PO.tile([1, C], F32, tag="pstat2", bufs=1, name=f"l1s2_{c}")
                    mid_cbs = dict(mid_cbs)
                    zsqs = {}
                    for n in range(NE):
                        wst = WS_.tile([P, NE, P], FP8, tag="wst8", name=f"wdst{c}_{n}")
                        nc.sync.dma_start(wst[:], wd_d[n])
                        ps = PO.tile([P, C], F32, tag="pmm", bufs=4, name=f"pwd{c}_{n}")
                        for kp in range(NE // 2):
                            nc.tensor.matmul(
                                ps[:],
                                wst[:, 2 * kp:2 * kp + 2, :],
                                astg8[:, 2 * kp:2 * kp + 2, :],
                                start=(kp == 0),
                                stop=(kp == NE // 2 - 1),
                                perf_mode=DR,
                            )
                        nc.vector.scalar_tensor_tensor(
                            z1[:, n, cs],
                            ps[:], DS,
                            xTc[:, n, cs],
                            op0=ALU.mult, op1=ALU.add,
                        )
                        zsq = R2.tile([P, C], BF16, tag="zsq", bufs=2, name=f"zsq1_{c}_{n}")
                        nc.scalar.activation(zsq[:], z1[:, n, cs], AF.Square)
                        # stats matmuls for tile n emitted after tile n+1's
                        # matmul chain so the PE never waits on the evac+
                        # square latency of tile n
                        if n >= 1:
                            _wd_stats(c, s1, s2, n - 1, zsqs[n - 1])
                        zsqs[n] = zsq
                        if n in mid_cbs:
                            mid_cbs[n]()
                    _wd_stats(c, s1, s2, NE - 1, zsqs[NE - 1])
                    sq_prewarm(f"wd{c}")
                    return s1, s2

                def _wd_stats(c, s1, s2, n, zsq):
                    cs = slice(c * C, (c + 1) * C)
                    nc.tensor.matmul(
                        s1[:], ones[:], z1[:, n, cs],
                        start=(n == 0), stop=(n == NE - 1),
                    )
                    nc.tensor.matmul(
                        s2[:], ones[:], zsq[:],
                        start=(n == 0), stop=(n == NE - 1),
                    )

                ffA = PP.tile([P, NF // 2, C], BF16, tag="ffA")
                ffB = PP.tile([P, NF // 2, C], BF16, tag="ffB")

                def ff1_stage(c, f, ffA, ffB):
                    cs = slice(c * C, (c + 1) * C)
                    wst8 = WS_.tile([P, 2, P], FP8, tag="wst8f", bufs=4)
                    nc.sync.dma_start(wst8[:], wi8_d[f])
                    wst = WS_.tile([P, NE - 2, P], BF16, tag="wst")
                    nc.sync.dma_start(wst[:], wi_d[f])
                    ps = PO.tile([P, C], F32, tag="pmm", bufs=4)
                    # k-tiles 0,1 as one fp8 DoubleRow matmul (h18 = 16*h1,
                    # wi8 = 32*Wi); k-tiles 2..7 in fp16 with Wi*512
                    nc.tensor.matmul(
                        ps[:], wst8[:], h18[:, :, cs],
                        start=True, stop=False, perf_mode=DR,
                    )
                    for k in range(2, NE):
                        nc.tensor.matmul(
                            ps[:],
                            wst[:, k - 2, :],
                            h1[:, k, cs],
                            start=False,
                            stop=(k == NE - 1),
                        )
                    dst = ffA if f < NF // 2 else ffB
                    nc.scalar.activation(
                        dst[:, f % (NF // 2), :], ps[:],
                        AF.Gelu, bias=bicol[:, f:f + 1], scale=DS,
                    )

                def ff2_mm_stats(c, z2, ffA, ffB, pfx, fs1, fs2, lo, w,
                                 mid_cbs=()):
                    """FF2 + bias + residual for cols [lo, lo+w) of chunk c
                    with inline LN2 stats into fs1/fs2 slices. Stats matmuls
                    for tile n are deferred past tile n+1's matmuls so the
                    PE never waits on the evac+square latency. mid_cbs maps
                    n -> callback emitted after that tile's matmul chain."""
                    zs = slice(lo, lo + w)
                    hs = slice(c * C + lo, c * C + lo + w)
                    mid_cbs = dict(mid_cbs)

                    def _stats(n, zsq):
                        nc.tensor.matmul(
                            fs1[0:1, lo:lo + w], ones[:], z2[:, n, zs],
                            start=(n == 0), stop=(n == NE - 1),
                        )
                        nc.tensor.matmul(
                            fs2[0:1, lo:lo + w], ones[:], zsq[:, :w],
                            start=(n == 0), stop=(n == NE - 1),
                        )

                    zsqs = {}
                    for n in range(NE):
                        pso = PO.tile([P, C], F32, tag="pmm", bufs=4, name=f"pso_{pfx}_{n}")
                        for g in range(4):
                            wst = WS_.tile([P, 8, P], BF16, tag="wst")
                            nc.sync.dma_start(wst[:], wo_d[n, g])
                            for j in range(8):
                                f = g * 8 + j
                                src = ffA if f < NF // 2 else ffB
                                nc.tensor.matmul(
                                    pso[:, :w],
                                    wst[:, j, :],
                                    src[:, f % (NF // 2), zs],
                                    start=(f == 0),
                                    stop=(f == NF - 1),
                                )
                        nc.vector.scalar_tensor_tensor(
                            z2[:, n, zs], pso[:, :w], bo[:, n:n + 1],
                            h1[:, n, hs],
                            op0=ALU.add, op1=ALU.add,
                        )
                        zsq = R2.tile([P, C], BF16, tag="zsq", bufs=2, name=f"zsq2_{pfx}_{n}")
                        nc.scalar.activation(zsq[:, :w], z2[:, n, zs], AF.Square)
                        if n >= 1:
                            _stats(n - 1, zsqs[n - 1])
                        zsqs[n] = zsq
                        if n in mid_cbs:
                            mid_cbs[n]()
                    _stats(NE - 1, zsqs[NE - 1])
                    sq_prewarm(f"ff2{pfx}")

                def ln2_finish(c, z2, bcax, bcbx, pfx, lo=0, w=C):
                    for n in range(NE):
                        oe = R2.tile([P, C], BF16, tag="outevac", bufs=4, name=f"oe_{pfx}_{n}")
                        t1 = R3.tile([P, C], BF16, tag="tmp", name=f"t1_{pfx}_{n}")
                        nc.vector.tensor_mul(t1[:, :w], z2[:, n, lo:lo + w], bcax[:, :w])
                        nc.vector.tensor_add(t1[:, :w], t1[:, :w], bcbx[:, :w])
                        if n % 2 == 0:
                            nc.scalar.activation(
                                oe[:, :w], t1[:, :w], AF.Identity,
                                bias=b2[:, n:n + 1], scale=g2[:, n:n + 1],
                            )
                        else:
                            nc.vector.scalar_tensor_tensor(
                                oe[:, :w], t1[:, :w], g2[:, n:n + 1],
                                b2[:, n:n + 1].to_broadcast([P, w]),
                                op0=ALU.mult, op1=ALU.add,
                            )
                        nc.sync.dma_start(
                            outT_d[n * P:(n + 1) * P, c * C + lo:c * C + lo + w],
                            oe[:, :w],
                        )

                # Pipeline: every LN scalar chain (DVE/scalar) is covered by
                # independent PE matmul work emitted just before it.
                s1_0, s2_0 = wd_chunk(0, astg0)
                bcax0, bcbx0 = _ln_scalars(
                    nc, SM, PO, s1_0[0:1, :], s2_0[0:1, :], E, epst,
                    ones_row, "l1c0"
                )
                s1_1, s2_1 = wd_chunk(1, astg1)
                for n in range(NE):
                    _ln_normalize(
                        nc, R3, z1[:, n, 0:C], bcax0, bcbx0, g1, b1, n,
                        h1[:, n, 0:C],
                    )
                    if n < 2:
                        nc.vector.tensor_scalar_mul(
                            h18[:, n, 0:C], h1[:, n, 0:C], XS
                        )
                # FF1(c0) f=0..2 fills LN1(c1)'s scalar chain
                for f in range(3):
                    ff1_stage(0, f, ffA, ffB)
                bcax1, bcbx1 = _ln_scalars(
                    nc, SM, PO, s1_1[0:1, :], s2_1[0:1, :], E, epst,
                    ones_row, "l1c1"
                )
                for f in range(3, 6):
                    ff1_stage(0, f, ffA, ffB)
                for n in range(NE):
                    _ln_normalize(
                        nc, R3, z1[:, n, C:2 * C], bcax1, bcbx1, g1, b1, n,
                        h1[:, n, C:2 * C],
                    )
                    if n < 2:
                        nc.vector.tensor_scalar_mul(
                            h18[:, n, C:2 * C], h1[:, n, C:2 * C], XS
                        )
                for f in range(6, NF):
                    ff1_stage(0, f, ffA, ffB)

                z2a = PP.tile([P, NE, C], BF16, tag="z2", name="z2_c0")
                fs1_0 = PO.tile([1, C], F32, tag="pstat1", bufs=1, name="fs1_c0")
                fs2_0 = PO.tile([1, C], F32, tag="pstat2", bufs=1, name="fs2_c0")
                ff2_mm_stats(0, z2a, ffA, ffB, "c0", fs1_0, fs2_0, 0, C)
                # FF1(c1) f=0..2 fills LN2(c0)'s scalar chain
                for f in range(3):
                    ff1_stage(1, f, ffA, ffB)
                l2x0, l2b0 = _ln_scalars(
                    nc, SM, PO, fs1_0[0:1, :], fs2_0[0:1, :], E, epst,
                    ones_row, "l2c0"
                )
                for f in range(3, NF):
                    ff1_stage(1, f, ffA, ffB)
                ln2_finish(0, z2a, l2x0, l2b0, "c0")

                # FF2(c1) stays full-width: a column-split would double the
                # wo staging rate past DMA bandwidth and starve the PE.
                z2b = PP.tile([P, NE, C], BF16, tag="z2", name="z2_c1")
                fs1_1 = PO.tile([1, C], F32, tag="pstat1", bufs=1, name="fs1_c1")
                fs2_1 = PO.tile([1, C], F32, tag="pstat2", bufs=1, name="fs2_c1")
                ff2_mm_stats(1, z2b, ffA, ffB, "c1", fs1_1, fs2_1, 0, C)
                l2x1, l2b1 = _ln_scalars(
                    nc, SM, PO, fs1_1[0:1, :], fs2_1[0:1, :], E, epst,
                    ones_row, "l2c1"
                )
                ln2_finish(1, z2b, l2x1, l2b1, "c1")
    nc.compile()
    return nc


_RUNNER_CACHE = None


def _get_runner():
    """Compile once; return f(in_maps) -> list[dict] using a cached jitted
    shard_map executable (8 cores, no donation so device buffers reuse)."""
    global _RUNNER_CACHE
    if _RUNNER_CACHE is not None:
        return _RUNNER_CACHE

    import jax
    from jax.sharding import Mesh, PartitionSpec
    from jax.experimental.shard_map import shard_map
    from concourse import bass2jax

    nc = build()
    bass2jax.install_neuronx_cc_hook()

    partition_name = (
        nc.partition_id_tensor.name if nc.partition_id_tensor else None
    )
    in_names, out_names, out_avals = [], [], []
    for alloc in nc.m.functions[0].allocations:
        if not isinstance(alloc, mybir.MemoryLocationSet):
            continue
        name = alloc.memorylocations[0].name
        if alloc.kind == "ExternalInput":
            if name != partition_name:
                in_names.append(name)
        elif alloc.kind == "ExternalOutput":
            out_names.append(name)
            out_avals.append(
                jax.core.ShapedArray(
                    tuple(alloc.tensor_shape), mybir.dt.np(alloc.dtype)
                )
            )
    n_params = len(in_names)
    all_in_names = in_names + out_names
    if partition_name is not None:
        all_in_names = all_in_names + [partition_name]

    def _body(*args):
        operands = list(args)
        if partition_name is not None:
            operands.append(bass2jax.partition_id_tensor())
        outs = bass2jax._bass_exec_p.bind(
            *operands,
            out_avals=tuple(out_avals),
            in_names=tuple(all_in_names),
            out_names=tuple(out_names),
            lowering_input_output_aliases=(),
            sim_require_finite=True,
            sim_require_nnan=True,
            nc=nc,
        )
        return tuple(outs)

    devices = jax.devices()[:B]
    mesh = Mesh(np.asarray(devices), ("core",))
    n_all = n_params + len(out_names)
    sharded = jax.jit(
        shard_map(
            _body,
            mesh=mesh,
            in_specs=(PartitionSpec("core"),) * n_all,
            out_specs=(PartitionSpec("core"),) * len(out_names),
            check_rep=False,
        ),
        keep_unused=True,
    )

    def run(in_maps, device_args=None, timing_reps=0):
        if device_args is None:
            concat_in = [
                np.concatenate([np.asarray(m[nm]) for m in in_maps], axis=0)
                for nm in in_names
            ]
            concat_zeros = [
                np.zeros((B * a.shape[0], *a.shape[1:]), a.dtype) for a in out_avals
            ]
            device_args = [jax.device_put(a) for a in concat_in + concat_zeros]
        out_arrs = sharded(*device_args)
        jax.block_until_ready(out_arrs)
        timings = []
        for _ in range(timing_reps):
            import time as _time

            t0 = _time.perf_counter()
            out_arrs = sharded(*device_args)
            jax.block_until_ready(out_arrs)
            timings.append(_time.perf_counter() - t0)
        results = [
            {
                nm: np.asarray(out_arrs[i]).reshape(B, *out_avals[i].shape)[c]
                for i, nm in enumerate(out_names)
            }
            for c in range(B)
        ]
        return results, device_args, timings

    _RUNNER_CACHE = run
    return run


def _pretile(w, nt, kt):
    """W [K, N] -> [nt, kt, 128, 128] with tile[n][k] = W[kblk, nblk]."""
    t = w.reshape(kt, P, nt, P).transpose(2, 0, 1, 3)
    return np.ascontiguousarray(t)


def _cols(vec):
    """[X*128] -> [128, X] with col j = vec[j*128:(j+1)*128]."""
    return np.ascontiguousarray(vec.reshape(-1, P).T)


def _build_in_maps(inputs):
    inp = {k: np.asarray(v, dtype=np.float32) for k, v in inputs.items()}
    x = inp["hidden_states"]  # [B, S, E]

    def _ptp(t):
        """[n, k, P, P] tile layout -> [n, P(p), k, P(m)] for contiguous DMA."""
        return np.ascontiguousarray(t.transpose(0, 2, 1, 3))

    wq = _ptp(_pretile(inp["Wq"] * WS, NE, NE)).astype(NPFP8)
    wk = _ptp(_pretile(inp["Wk"] * WS, NE, NE)).astype(NPFP8)
    wd = _ptp(_pretile(inp["Wd"] * WS, NE, NE)).astype(NPFP8)
    wi_t = _ptp(_pretile(inp["Wi"], NF, NE))  # [NF, P, NE, P]
    wi8 = np.ascontiguousarray(wi_t[:, :, 0:2, :] * WS).astype(NPFP8)
    wi = np.ascontiguousarray(wi_t[:, :, 2:, :] * (XS * WS)).astype(NPBF16)
    wo = np.ascontiguousarray(
        _pretile(inp["Wo"], NE, NF).reshape(NE, 4, 8, P, P).transpose(0, 1, 3, 2, 4)
    ).astype(NPBF16)
    # wv [NC, NE//2, P, 2, C]: wv[c, kp, p, i, j] = Wv[(2kp+i)P + p, cC + j]
    wv = (inp["Wv"] * WS).reshape(NE // 2, 2, P, NC, C)
    wv = np.ascontiguousarray(wv.transpose(3, 0, 2, 1, 4)).astype(NPFP8)

    zero = np.zeros_like(inp["bq"])
    bias_full = np.concatenate(
        [
            _cols(inp["bq"]), _cols(inp["bk"]),
            _cols(zero), _cols(zero),
            _cols(inp["g1"]), _cols(inp["b1"]),
            _cols(inp["bo"]), _cols(inp["g2"]), _cols(inp["b2"]),
        ],
        axis=1,
    )
    bicol = _cols(inp["bi"])
    c_fold = inp["bd"] + inp["bv"] @ inp["Wd"]  # LN1 residual bias

    in_maps = []
    for bidx in range(B):
        xT = np.ascontiguousarray(x[bidx].T)  # [E, S] f32
        xT8 = (xT * XS).astype(NPFP8)
        xTc = (xT + c_fold[:, None]).astype(NPBF16)
        in_maps.append(
            {
                "xT8": xT8, "xTc": xTc, "wq": wq, "wk": wk, "wv": wv,
                "wd": wd, "wi": wi, "wi8": wi8, "wo": wo, "biases": bias_full,
                "bi_cols": bicol,
                "ones_in": np.ones((P, 1), dtype=NPBF16),
                "ones8_in": np.ones((P, 1), dtype=NPFP8),
                "onesrow_in": np.ones((1, P), dtype=NPBF16),
            }
        )
    return in_maps


def kernel(**inputs):
    run = _get_runner()
    results, _, _ = run(_build_in_maps(inputs))
    out = np.stack([r["outT"].T for r in results]).astype(np.float32)
    return out


# revision 74
# speedup vs baseline: 1.0223x; 1.0223x over previous
"""BERT layer (B=8, S=1024, E=1024, F=4096) on 8 trn2 NeuronCores.

Strategy: pure data-parallel over batch (1 element per core, no collectives).
Per-core kernel keeps activations feature-major ([features, tokens]) so every
weight matmul uses the natural [in, out] weight block as the PE stationary
operand.

The attention path (QKV projections, scores, AV, Wd) runs on the PE in
fp8-e4m3 with DoubleRow perf mode (2x the bf16 rate, measured on HW);
accumulation stays fp32 in PSUM, so only operand-quantization noise is
added (1.73e-2 max rel err measured vs the 2e-2 gate). The FF block
(h1@Wi, ff@Wo) dominates the remaining flops but is too error-sensitive for
full fp8 (activation-quantization noise feeds the output LN undiluted), so
it stays fp16 except the first quarter of FF1's contraction range, which
runs fp8-DoubleRow. LayerNorm statistics run on the PE via ones-vector matmuls
interleaved into the producing loops; residuals and LN chains stay
fp16/fp32. Scale folding: x is staged as fp8(16*x), weights as fp8(32*W),
descale (1/512) is folded into the PSUM-evacuation activation; the LN1
residual+bias (x + bd + bv@Wd) is folded into the fp16 copy of x on the
host; exp runs as exp(scores/32 - 2) so fp8 attention weights stay inside
e4m3 range, the -2 cancelling in the softmax normalization.
"""

import sys

for _p in ("/opt/trn_rl_repo", "/root/.axon_site/_ro/trn_rl_repo"):
    if _p not in sys.path:
        sys.path.append(_p)

import numpy as np
import ml_dtypes

import concourse.bass as bass  # noqa: F401
import concourse.mybir as mybir
from concourse import bacc
from concourse.bass_utils import run_bass_kernel_spmd  # noqa: F401
from concourse.tile import TileContext

B, S, E, F = 8, 1024, 1024, 4096
P = 128
NE = E // P     # 8 tiles along E
NF = F // P     # 32 tiles along F
NS = S // P     # 8 tiles along S
C = 512         # free-dim chunk (one fp32 psum bank)
NC = S // C     # 2 chunks along S
EPS = 1e-12
AF = mybir.ActivationFunctionType
ALU = mybir.AluOpType
F32 = mybir.dt.float32
BF16 = mybir.dt.float16
FP8 = mybir.dt.float8e4
DR = mybir.MatmulPerfMode.DoubleRow
NPBF16 = mybir.dt.np(BF16)
NPFP8 = ml_dtypes.float8_e4m3

XS = 16.0        # fp8 scale on x
WS = 32.0        # fp8 scale on attention-path weights
DS = 1.0 / (XS * WS)   # descale after x8 @ w8 psum


def _ln_scalars(nc, SM, psum_pool, s1, s2, dim, epst, ones_row, pfx, w=C):
    """From column-sum psum APs s1=sum(z), s2=sum(z^2) [1,w] build two fp16
    SBUF broadcast tiles [P,w] (as [:, :w] views of full-width scratch):
    bcax = rstd, bcbx = -mu*rstd."""
    negmu = SM.tile([1, C], F32, tag="lnt0", name=f"negmu_{pfx}")
    musq = SM.tile([1, C], F32, tag="lnt1", name=f"musq_{pfx}")
    var = SM.tile([1, C], F32, tag="lnt2", name=f"var_{pfx}")
    std = SM.tile([1, C], F32, tag="lnt1", name=f"std_{pfx}")
    rstd = SM.tile([1, C], F32, tag="lnt2", name=f"rstd_{pfx}")
    nmr = SM.tile([1, C], BF16, tag="lnt3", name=f"nmr_{pfx}")
    rstd_r = SM.tile([1, C], BF16, tag="lnt4", name=f"rstdr_{pfx}")
    nc.vector.tensor_scalar_mul(negmu[:, :w], s1, -1.0 / dim)
    nc.vector.tensor_tensor(musq[:, :w], negmu[:, :w], negmu[:, :w], op=ALU.mult)
    nc.vector.scalar_tensor_tensor(
        var[:, :w], s2, 1.0 / dim, musq[:, :w], op0=ALU.mult, op1=ALU.subtract
    )
    nc.scalar.activation(std[:, :w], var[:, :w], AF.Sqrt, bias=epst[0:1, 0:1])
    nc.vector.reciprocal_approx_fast(out=rstd[:, :w], in_=std[:, :w])
    nc.vector.tensor_tensor(nmr[:, :w], negmu[:, :w], rstd[:, :w], op=ALU.mult)
    nc.vector.tensor_copy(rstd_r[:, :w], rstd[:, :w])
    bca = psum_pool.tile([P, C], F32, tag="bca", bufs=1, name=f"bca_{pfx}")
    bcb = psum_pool.tile([P, C], F32, tag="bcb", bufs=1, name=f"bcb_{pfx}")
    nc.tensor.matmul(bca[:, :w], ones_row[:], rstd_r[:, :w], start=True, stop=True)
    nc.tensor.matmul(bcb[:, :w], ones_row[:], nmr[:, :w], start=True, stop=True)
    bcax = SM.tile([P, C], BF16, tag="bcax", name=f"bcax_{pfx}")
    bcbx = SM.tile([P, C], BF16, tag="bcbx", name=f"bcbx_{pfx}")
    nc.vector.tensor_copy(bcax[:, :w], bca[:, :w])
    nc.vector.tensor_copy(bcbx[:, :w], bcb[:, :w])
    return bcax, bcbx


def _ln_normalize(nc, R3, z_tile, bcax, bcbx, g, b, n, dst_ap):
    """dst = ((z * rstd) + (-mu*rstd)) * g[n] + b[n] for one [P, C] tile."""
    t1 = R3.tile([P, C], BF16, tag="tmp")
    nc.vector.tensor_mul(t1[:], z_tile, bcax[:])
    nc.vector.tensor_add(t1[:], t1[:], bcbx[:])
    nc.scalar.activation(
        dst_ap, t1[:], AF.Identity, bias=b[:, n:n + 1], scale=g[:, n:n + 1]
    )


def build():
    nc = bacc.Bacc("TRN2", target_bir_lowering=False, debug=False)

    xT8_d = nc.dram_tensor("xT8", [E, S], FP8, kind="ExternalInput")
    xTc_d = nc.dram_tensor("xTc", [E, S], BF16, kind="ExternalInput")
    # all weight tiles stored pre-transposed so every staging DMA is a
    # contiguous per-partition copy (the strided k->p rearrange DMAs moved
    # to host numpy)
    wq_d = nc.dram_tensor("wq", [NE, P, NE, P], FP8, kind="ExternalInput")
    wk_d = nc.dram_tensor("wk", [NE, P, NE, P], FP8, kind="ExternalInput")
    wv_d = nc.dram_tensor(
        "wv", [NC, NE // 2, P, 2, C], FP8, kind="ExternalInput"
    )
    wd_d = nc.dram_tensor("wd", [NE, P, NE, P], FP8, kind="ExternalInput")
    # FF1 runs its first 2 k-tiles (256 of 1024 contraction rows) in fp8
    # DoubleRow and the rest in fp16 -- partial-K quantization keeps the
    # added LN2 noise inside the error budget while saving 1 of 8 chain
    # matmuls. The fp16 weight half carries the 512x scale of the fp8 path
    # (folded on host) so both accumulate into one psum chain.
    wi8_d = nc.dram_tensor("wi8", [NF, P, 2, P], FP8, kind="ExternalInput")
    wi_d = nc.dram_tensor("wi", [NF, P, NE - 2, P], BF16, kind="ExternalInput")
    wo_d = nc.dram_tensor("wo", [NE, 4, P, 8, P], BF16, kind="ExternalInput")
    # bias columns: [bq, bk, -, -, g1, b1, bo, g2, b2] -> [P, 9*NE]
    bias_d = nc.dram_tensor("biases", [P, 9 * NE], F32, kind="ExternalInput")
    bi_d = nc.dram_tensor("bi_cols", [P, NF], F32, kind="ExternalInput")
    ones_d = nc.dram_tensor("ones_in", [P, 1], BF16, kind="ExternalInput")
    ones8_d = nc.dram_tensor("ones8_in", [P, 1], FP8, kind="ExternalInput")
    onesrow_d = nc.dram_tensor("onesrow_in", [1, P], BF16, kind="ExternalInput")
    # fp16 output (host upconverts): strictly less work -- halves the
    # tail out-DMA and drain; correctness verified on HW (rel err
    # unchanged at 1.7327e-2)
    outT_d = nc.dram_tensor("outT", [E, S], BF16, kind="ExternalOutput")

    with TileContext(nc) as tc:
        with (
            tc.tile_pool(name="persist", bufs=1) as PP,
            tc.tile_pool(name="wstage", bufs=4) as WS_,
            tc.tile_pool(name="small", bufs=1) as SM,
            tc.tile_pool(name="rot3", bufs=6) as R3,
            tc.tile_pool(name="rot2", bufs=2) as R2,
        ):
            # ---- x staged fp8 (matmuls) + fp16 with LN1 bias folded ----
            xT8 = PP.tile([P, NE, S], FP8, tag="xT8")
            xTc = PP.tile([P, NE, S], BF16, tag="xTc")
            # stage kp=0's operands issue first (each dma_start costs
            # ~0.6us of Sync-engine issue time), then kp=1's
            wvst0 = WS_.tile([P, 2, C], FP8, tag="wvst", bufs=3, name="wvst00")
            wvst1 = WS_.tile([P, 2, C], FP8, tag="wvst", bufs=3, name="wvst01")
            nc.sync.dma_start(wvst0[:], wv_d[0, 0])
            for k in range(2):
                nc.sync.dma_start(xT8[:, k, :], xT8_d[k * P:(k + 1) * P, :])
            nc.sync.dma_start(wvst1[:], wv_d[0, 1])
            for k in range(2, 4):
                nc.sync.dma_start(xT8[:, k, :], xT8_d[k * P:(k + 1) * P, :])

            # ================= v = x @ Wv (token-major, no bias) ============
            v8 = PP.tile([P, NS, E], FP8, tag="v8")
            with tc.tile_pool(name="pv", bufs=1, space="PSUM") as PV:
                for c in range(NC):
                    pvs = [
                        PV.tile([P, C], F32, tag=f"pv{s_t}", name=f"pv{s_t}_{c}")
                        for s_t in range(NS)
                    ]
                    for kp in range(NE // 2):
                        if c == 0 and kp > 1:
                            for k in (2 * kp, 2 * kp + 1):
                                nc.sync.dma_start(
                                    xT8[:, k, :], xT8_d[k * P:(k + 1) * P, :]
                                )
                        if c == 0 and kp == 0:
                            wvst = wvst0
                        elif c == 0 and kp == 1:
                            wvst = wvst1
                        else:
                            wvst = WS_.tile([P, 2, C], FP8, tag="wvst", bufs=3)
                            nc.sync.dma_start(wvst[:], wv_d[c, kp])
                        for s_t in range(NS):
                            nc.tensor.matmul(
                                pvs[s_t][:],
                                xT8[:, 2 * kp:2 * kp + 2, s_t * P:(s_t + 1) * P],
                                wvst[:],
                                start=(kp == 0),
                                stop=(kp == NE // 2 - 1),
                                perf_mode=DR,
                            )
                    for s_t in range(NS):
                        # v8 = fp8(16 * v) = ps * (16 * DS); alternate the
                        # evacs between scalar and vector engines so the
                        # psum WAR chain for chunk c+1 clears 2x faster
                        if s_t % 2 == 0:
                            nc.scalar.activation(
                                v8[:, s_t, c * C:(c + 1) * C], pvs[s_t][:],
                                AF.Identity, scale=XS * DS,
                            )
                        else:
                            nc.vector.tensor_scalar_mul(
                                v8[:, s_t, c * C:(c + 1) * C], pvs[s_t][:],
                                XS * DS,
                            )

            # ---- constants ----
            ones = SM.tile([P, 1], BF16, tag="ones")
            nc.sync.dma_start(ones[:], ones_d[:])
            ones8 = SM.tile([P, 1], FP8, tag="ones8")
            nc.sync.dma_start(ones8[:], ones8_d[:])
            ones_row = SM.tile([1, P], BF16, tag="onesrow")
            nc.sync.dma_start(ones_row[:], onesrow_d[:])
            epst = SM.tile([1, 1], F32, tag="epst")
            nc.vector.memset(epst[:], EPS)
            neg2 = SM.tile([P, 1], F32, tag="neg2")
            nc.vector.memset(neg2[:], -2.0)
            biases = SM.tile([P, 9 * NE], F32, tag="biases")
            nc.sync.dma_start(biases[:], bias_d[:])
            bq = biases[:, 0 * NE:1 * NE]
            bk = biases[:, 1 * NE:2 * NE]
            g1 = biases[:, 4 * NE:5 * NE]
            b1 = biases[:, 5 * NE:6 * NE]
            bo = biases[:, 6 * NE:7 * NE]
            g2 = biases[:, 7 * NE:8 * NE]
            b2 = biases[:, 8 * NE:9 * NE]
            bicol = SM.tile([P, NF], F32, tag="bicol")
            nc.sync.dma_start(bicol[:], bi_d[:])

            # ================= qT / kT (fp8 out, bias added) ================
            qT8 = PP.tile([P, NE, S], FP8, tag="qT8")
            kT8 = PP.tile([P, NE, S], FP8, tag="kT8")
            with tc.tile_pool(name="pqk", bufs=3, space="PSUM") as PQK:
                for wi_, (w_d, dst, bias_ap) in enumerate(
                    ((wq_d, qT8, bq), (wk_d, kT8, bk))
                ):
                    for n in range(NE):
                        wst = WS_.tile([P, NE, P], FP8, tag="wst8")
                        nc.sync.dma_start(wst[:], w_d[n])
                        if wi_ == 1:
                            # spread the xTc residual prefetch (2 MB) across
                            # the SECOND projection's weight stages: the
                            # first stages after the v->qk transition keep a
                            # clean DMA queue for wst8 prefetch
                            nc.sync.dma_start(
                                xTc[:, n, :], xTc_d[n * P:(n + 1) * P, :]
                            )
                        for c in range(NC):
                            ps = PQK.tile([P, C], F32, tag="pqk", bufs=6)
                            for kp in range(NE // 2):
                                nc.tensor.matmul(
                                    ps[:],
                                    wst[:, 2 * kp:2 * kp + 2, :],
                                    xT8[:, 2 * kp:2 * kp + 2, c * C:(c + 1) * C],
                                    start=(kp == 0),
                                    stop=(kp == NE // 2 - 1),
                                    perf_mode=DR,
                                )
                            # alternate evac engines (scalar act / DVE STT)
                            if (2 * n + c) % 2 == 0:
                                nc.scalar.activation(
                                    dst[:, n, c * C:(c + 1) * C], ps[:],
                                    AF.Identity,
                                    bias=bias_ap[:, n:n + 1], scale=DS,
                                )
                            else:
                                nc.vector.scalar_tensor_tensor(
                                    dst[:, n, c * C:(c + 1) * C], ps[:], DS,
                                    bias_ap[:, n:n + 1].to_broadcast([P, C]),
                                    op0=ALU.mult, op1=ALU.add,
                                )

            # ================= attention ====================================
            z1 = PP.tile([P, NE, S], BF16, tag="z1")
            h1 = PP.tile([P, NE, S], BF16, tag="h1")
            h18 = PP.tile([P, 2, S], FP8, tag="h18")  # fp8(16*h1), k-tiles 0,1

            ATT_B = tc.tile_pool(name="attB", bufs=1, space="PSUM")
            PB = ATT_B.__enter__()
            ATT_A = tc.tile_pool(name="attA", bufs=1, space="PSUM")
            PA = ATT_A.__enter__()

            def attn_scores(qb):
                qs = slice(qb * C, (qb + 1) * C)
                wT8 = PP.tile([P, NS, C], FP8, tag="wT8", name=f"wT8_{qb}")
                for j in range(NS):
                    ps_sT = PA.tile(
                        [P, C], F32, tag="psc", bufs=3, name=f"ps_sT{qb}_{j}"
                    )
                    for kp in range(NE // 2):
                        nc.tensor.matmul(
                            ps_sT[:],
                            kT8[:, 2 * kp:2 * kp + 2, j * P:(j + 1) * P],
                            qT8[:, 2 * kp:2 * kp + 2, qs],
                            start=(kp == 0),
                            stop=(kp == NE // 2 - 1),
                            perf_mode=DR,
                        )
                    # wT = exp(scores/32 - 2); -2 cancels in normalization,
                    # keeps fp8 range (max exp(5.3-2) ~ 27 << 240)
                    nc.scalar.activation(
                        wT8[:, j, :], ps_sT[:], AF.Exp,
                        scale=1.0 / 32.0, bias=neg2[:, 0:1],
                    )
                return wT8

            def attn_den(qb, wT8):
                ps_den = PA.tile([1, C], F32, tag="pden", bufs=1, name=f"psden{qb}")
                for j in range(NS):
                    # dual-fp8 LDWEIGHTS rejects free-size-2 weights, so the
                    # denominator reduction stays non-DoubleRow (fp8 at 1 cyc)
                    nc.tensor.matmul(
                        ps_den[:], ones8[:], wT8[:, j, :],
                        start=(j == 0), stop=(j == NS - 1),
                    )
                rec = SM.tile([1, C], F32, tag="rcp", name=f"rec{qb}")
                nc.vector.reciprocal_approx_fast(out=rec[:], in_=ps_den[:])
                rec_r = SM.tile([1, C], BF16, tag="rcpr", name=f"recr{qb}")
                nc.vector.tensor_copy(rec_r[:], rec[:])
                return rec_r

            def attn_bcast(qb, rec_r):
                bcq = PA.tile([P, C], F32, tag="bcq", bufs=1, name=f"bcq{qb}")
                nc.tensor.matmul(
                    bcq[:], ones_row[:], rec_r[:], start=True, stop=True,
                )
                recb = SM.tile([P, C], F32, tag="rcb", name=f"recb{qb}")
                nc.vector.tensor_copy(recb[:], bcq[:])
                return recb

            def attn_av(qb, wT8, recb):
                # astg8 = fp8(16*attn) = fp8(ps_a * (1/den)); the 16 comes
                # from v8 already carrying 16*v
                astg8 = PP.tile(
                    [P, NE, C], FP8, tag="astg8", bufs=2, name=f"astg8_{qb}"
                )
                for e_t in range(NE):
                    ps_a = PB.tile(
                        [P, C], F32, tag="pav", bufs=3, name=f"ps_a{qb}_{e_t}"
                    )
                    for jp in range(NS // 2):
                        nc.tensor.matmul(
                            ps_a[:],
                            v8[:, 2 * jp:2 * jp + 2, e_t * P:(e_t + 1) * P],
                            wT8[:, 2 * jp:2 * jp + 2, :],
                            start=(jp == 0),
                            stop=(jp == NS // 2 - 1),
                            perf_mode=DR,
                        )
                    nc.vector.tensor_mul(astg8[:, e_t, :], ps_a[:], recb[:])
                return astg8

            # Interleave the two query blocks so the PE never waits on the
            # DVE reciprocal chains: block-1 scores fill block-0's rec
            # latency, and rec1 computes during bcast0 + av0.
            wT0 = attn_scores(0)
            rec_r0 = attn_den(0, wT0)
            wT1 = attn_scores(1)
            rec_r1 = attn_den(1, wT1)
            recb0 = attn_bcast(0, rec_r0)
            astg0 = attn_av(0, wT0, recb0)
            recb1 = attn_bcast(1, rec_r1)
            astg1 = attn_av(1, wT1, recb1)
            ATT_A.__exit__(None, None, None)
            ATT_B.__exit__(None, None, None)

            # ====== post-attention scope: Wd + LN1 + FF + LN2 ==============
            with tc.tile_pool(name="postpsum", bufs=1, space="PSUM") as PO:

                def sq_prewarm(pfx):
                    """Load the Sqrt table right after the last Square of a
                    stats loop, so the LN scalar chain's Sqrt needs no
                    ACT_TABLE_LOAD on the critical path."""
                    sqw = SM.tile([1, 1], F32, tag="sqwarm", name=f"sqw_{pfx}")
                    nc.scalar.activation(sqw[:], epst[:], AF.Sqrt)

                def wd_chunk(c, astg8, mid_cbs=()):
                    """Wd matmuls + z1 evac + inline LN1 stats for chunk c.
                    z1 = ps * (1/512) + (x + bd + bv@Wd)   [bias folded on
                    host into xTc; astg carries 16x, wd carries 32x]."""
                    cs = slice(c * C, (c + 1) * C)
                    s1 = PO.tile([1, C], F32, tag="pstat1", bufs=1, name=f"l1s1_{c}")
                    s2 = PO.tile([1, C], F32, tag="pstat2", bufs=1, name=f"l1s2_{c}")
                    mid_cbs = dict(mid_cbs)
                    zsqs = {}
                    for n in range(NE):
                        wst = WS_.tile([P, NE, P], FP8, tag="wst8", name=f"wdst{c}_{n}")
                        nc.sync.dma_start(wst[:], wd_d[n])
                        ps = PO.tile([P, C], F32, tag="pmm", bufs=4, name=f"pwd{c}_{n}")
                        for kp in range(NE // 2):
                            nc.tensor.matmul(
                                ps[:],
                                wst[:, 2 * kp:2 * kp + 2, :],
                                astg8[:, 2 * kp:2 * kp + 2, :],
                                start=(kp == 0),
                                stop=(kp == NE // 2 - 1),
                                perf_mode=DR,
                            )
                        nc.vector.scalar_tensor_tensor(
                            z1[:, n, cs],
                            ps[:], DS,
                            xTc[:, n, cs],
                            op0=ALU.mult, op1=ALU.add,
                        )
                        zsq = R2.tile([P, C], BF16, tag="zsq", bufs=2, name=f"zsq1_{c}_{n}")
                        nc.scalar.activation(zsq[:], z1[:, n, cs], AF.Square)
                        # stats matmuls for tile n emitted after tile n+1's
                        # matmul chain so the PE never waits on the evac+
                        # square latency of tile n
                        if n >= 1:
                            _wd_stats(c, s1, s2, n - 1, zsqs[n - 1])
                        zsqs[n] = zsq
                        if n in mid_cbs:
                            mid_cbs[n]()
                    _wd_stats(c, s1, s2, NE - 1, zsqs[NE - 1])
                    sq_prewarm(f"wd{c}")
                    return s1, s2

                def _wd_stats(c, s1, s2, n, zsq):
                    cs = slice(c * C, (c + 1) * C)
                    nc.tensor.matmul(
                        s1[:], ones[:], z1[:, n, cs],
                        start=(n == 0), stop=(n == NE - 1),
                    )
                    nc.tensor.matmul(
                        s2[:], ones[:], zsq[:],
                        start=(n == 0), stop=(n == NE - 1),
                    )

                ffA = PP.tile([P, NF // 2, C], BF16, tag="ffA")
                ffB = PP.tile([P, NF // 2, C], BF16, tag="ffB")

                def ff1_stage(c, f, ffA, ffB):
                    cs = slice(c * C, (c + 1) * C)
                    wst8 = WS_.tile([P, 2, P], FP8, tag="wst8f", bufs=4)
                    nc.sync.dma_start(wst8[:], wi8_d[f])
                    wst = WS_.tile([P, NE - 2, P], BF16, tag="wst")
                    nc.sync.dma_start(wst[:], wi_d[f])
                    ps = PO.tile([P, C], F32, tag="pmm", bufs=4)
                    # k-tiles 0,1 as one fp8 DoubleRow matmul (h18 = 16*h1,
                    # wi8 = 32*Wi); k-tiles 2..7 in fp16 with Wi*512
                    nc.tensor.matmul(
                        ps[:], wst8[:], h18[:, :, cs],
                        start=True, stop=False, perf_mode=DR,
                    )
                    for k in range(2, NE):
                        nc.tensor.matmul(
                            ps[:],
                            wst[:, k - 2, :],
                            h1[:, k, cs],
                            start=False,
                            stop=(k == NE - 1),
                        )
                    dst = ffA if f < NF // 2 else ffB
                    nc.scalar.activation(
                        dst[:, f % (NF // 2), :], ps[:],
                        AF.Gelu, bias=bicol[:, f:f + 1], scale=DS,
                    )

                def ff2_mm_stats(c, z2, ffA, ffB, pfx, fs1, fs2, lo, w,
                                 mid_cbs=()):
                    """FF2 + bias + residual for cols [lo, lo+w) of chunk c
                    with inline LN2 stats into fs1/fs2 slices. Stats matmuls
                    for tile n are deferred past tile n+1's matmuls so the
                    PE never waits on the evac+square latency. mid_cbs maps
                    n -> callback emitted after that tile's matmul chain."""
                    zs = slice(lo, lo + w)
                    hs = slice(c * C + lo, c * C + lo + w)
                    mid_cbs = dict(mid_cbs)

                    def _stats(n, zsq):
                        nc.tensor.matmul(
                            fs1[0:1, lo:lo + w], ones[:], z2[:, n, zs],
                            start=(n == 0), stop=(n == NE - 1),
                        )
                        nc.tensor.matmul(
                            fs2[0:1, lo:lo + w], ones[:], zsq[:, :w],
                            start=(n == 0), stop=(n == NE - 1),
                        )

                    zsqs = {}
                    for n in range(NE):
                        pso = PO.tile([P, C], F32, tag="pmm", bufs=4, name=f"pso_{pfx}_{n}")
                        for g in range(4):
                            wst = WS_.tile([P, 8, P], BF16, tag="wst")
                            nc.sync.dma_start(wst[:], wo_d[n, g])
                            for j in range(8):
                                f = g * 8 + j
                                src = ffA if f < NF // 2 else ffB
                                nc.tensor.matmul(
                                    pso[:, :w],
                                    wst[:, j, :],
                                    src[:, f % (NF // 2), zs],
                                    start=(f == 0),
                                    stop=(f == NF - 1),
                                )
                        nc.vector.scalar_tensor_tensor(
                            z2[:, n, zs], pso[:, :w], bo[:, n:n + 1],
                            h1[:, n, hs],
                            op0=ALU.add, op1=ALU.add,
                        )
                        zsq = R2.tile([P, C], BF16, tag="zsq", bufs=2, name=f"zsq2_{pfx}_{n}")
                        nc.scalar.activation(zsq[:, :w], z2[:, n, zs], AF.Square)
                        if n >= 1:
                            _stats(n - 1, zsqs[n - 1])
                        zsqs[n] = zsq
                        if n in mid_cbs:
                            mid_cbs[n]()
                    _stats(NE - 1, zsqs[NE - 1])
                    sq_prewarm(f"ff2{pfx}")

                def ln2_finish(c, z2, bcax, bcbx, pfx, lo=0, w=C):
                    for n in range(NE):
                        oe = R2.tile([P, C], BF16, tag="outevac", bufs=4, name=f"oe_{pfx}_{n}")
                        t1 = R3.tile([P, C], BF16, tag="tmp", name=f"t1_{pfx}_{n}")
                        nc.vector.tensor_mul(t1[:, :w], z2[:, n, lo:lo + w], bcax[:, :w])
                        nc.vector.tensor_add(t1[:, :w], t1[:, :w], bcbx[:, :w])
                        if n % 2 == 0:
                            nc.scalar.activation(
                                oe[:, :w], t1[:, :w], AF.Identity,
                                bias=b2[:, n:n + 1], scale=g2[:, n:n + 1],
                            )
                        else:
                            nc.vector.scalar_tensor_tensor(
                                oe[:, :w], t1[:, :w], g2[:, n:n + 1],
                                b2[:, n:n + 1].to_broadcast([P, w]),
                                op0=ALU.mult, op1=ALU.add,
                            )
                        nc.sync.dma_start(
                            outT_d[n * P:(n + 1) * P, c * C + lo:c * C + lo + w],
                            oe[:, :w],
                        )

                # Pipeline: every LN scalar chain (DVE/scalar) is covered by
                # independent PE matmul work emitted just before it.
                s1_0, s2_0 = wd_chunk(0, astg0)
                bcax0, bcbx0 = _ln_scalars(
                    nc, SM, PO, s1_0[0:1, :], s2_0[0:1, :], E, epst,
                    ones_row, "l1c0"
                )
                s1_1, s2_1 = wd_chunk(1, astg1)
                for n in range(NE):
                    _ln_normalize(
                        nc, R3, z1[:, n, 0:C], bcax0, bcbx0, g1, b1, n,
                        h1[:, n, 0:C],
                    )
                    if n < 2:
                        nc.vector.tensor_scalar_mul(
                            h18[:, n, 0:C], h1[:, n, 0:C], XS
                        )
                # FF1(c0) f=0..2 fills LN1(c1)'s scalar chain
                for f in range(3):
                    ff1_stage(0, f, ffA, ffB)
                bcax1, bcbx1 = _ln_scalars(
                    nc, SM, PO, s1_1[0:1, :], s2_1[0:1, :], E, epst,
                    ones_row, "l1c1"
                )
                for f in range(3, 6):
                    ff1_stage(0, f, ffA, ffB)
                for n in range(NE):
                    _ln_normalize(
                        nc, R3, z1[:, n, C:2 * C], bcax1, bcbx1, g1, b1, n,
                        h1[:, n, C:2 * C],
                    )
                    if n < 2:
                        nc.vector.tensor_scalar_mul(
                            h18[:, n, C:2 * C], h1[:, n, C:2 * C], XS
                        )
                for f in range(6, NF):
                    ff1_stage(0, f, ffA, ffB)

                z2a = PP.tile([P, NE, C], BF16, tag="z2", name="z2_c0")
                fs1_0 = PO.tile([1, C], F32, tag="pstat1", bufs=1, name="fs1_c0")
                fs2_0 = PO.tile([1, C], F32, tag="pstat2", bufs=1, name="fs2_c0")
                ff2_mm_stats(0, z2a, ffA, ffB, "c0", fs1_0, fs2_0, 0, C)
                # FF1(c1) f=0..2 fills LN2(c0)'s scalar chain
                for f in range(3):
                    ff1_stage(1, f, ffA, ffB)
                l2x0, l2b0 = _ln_scalars(
                    nc, SM, PO, fs1_0[0:1, :], fs2_0[0:1, :], E, epst,
                    ones_row, "l2c0"
                )
                for f in range(3, NF):
                    ff1_stage(1, f, ffA, ffB)
                ln2_finish(0, z2a, l2x0, l2b0, "c0")

                # FF2(c1) stays full-width: a column-split would double the
                # wo staging rate past DMA bandwidth and starve the PE.
                z2b = PP.tile([P, NE, C], BF16, tag="z2", name="z2_c1")
                fs1_1 = PO.tile([1, C], F32, tag="pstat1", bufs=1, name="fs1_c1")
                fs2_1 = PO.tile([1, C], F32, tag="pstat2", bufs=1, name="fs2_c1")
                ff2_mm_stats(1, z2b, ffA, ffB, "c1", fs1_1, fs2_1, 0, C)
                l2x1, l2b1 = _ln_scalars(
                    nc, SM, PO, fs1_1[0:1, :], fs2_1[0:1, :], E, epst,
                    ones_row, "l2c1"
                )
                ln2_finish(1, z2b, l2x1, l2b1, "c1")
    nc.compile()
    return nc


_RUNNER_CACHE = None


def _get_runner():
    """Compile once; return f(in_maps) -> list[dict] using a cached jitted
    shard_map executable (8 cores, no donation so device buffers reuse)."""
    global _RUNNER_CACHE
    if _RUNNER_CACHE is not None:
        return _RUNNER_CACHE

    import jax
    from jax.sharding import Mesh, PartitionSpec
    from jax.experimental.shard_map import shard_map
    from concourse import bass2jax

    nc = build()
    bass2jax.install_neuronx_cc_hook()

    partition_name = (
        nc.partition_id_tensor.name if nc.partition_id_tensor else None
    )
    in_names, out_names, out_avals = [], [], []
    for alloc in nc.m.functions[0].allocations:
        if not isinstance(alloc, mybir.MemoryLocationSet):
            continue
        name = alloc.memorylocations[0].name
        if alloc.kind == "ExternalInput":
            if name != partition_name:
                in_names.append(name)
        elif alloc.kind == "ExternalOutput":
            out_names.append(name)
            out_avals.append(
                jax.core.ShapedArray(
                    tuple(alloc.tensor_shape), mybir.dt.np(alloc.dtype)
                )
            )
    n_params = len(in_names)
    all_in_names = in_names + out_names
    if partition_name is not None:
        all_in_names = all_in_names + [partition_name]

    def _body(*args):
        operands = list(args)
        if partition_name is not None:
            operands.append(bass2jax.partition_id_tensor())
        outs = bass2jax._bass_exec_p.bind(
            *operands,
            out_avals=tuple(out_avals),
            in_names=tuple(all_in_names),
            out_names=tuple(out_names),
            lowering_input_output_aliases=(),
            sim_require_finite=True,
            sim_require_nnan=True,
            nc=nc,
        )
        return tuple(outs)

    devices = jax.devices()[:B]
    mesh = Mesh(np.asarray(devices), ("core",))
    n_all = n_params + len(out_names)
    sharded = jax.jit(
        shard_map(
            _body,
            mesh=mesh,
            in_specs=(PartitionSpec("core"),) * n_all,
            out_specs=(PartitionSpec("core"),) * len(out_names),
            check_rep=False,
        ),
        keep_unused=True,
    )

    def run(in_maps, device_args=None, timing_reps=0):
        if device_args is None:
            concat_in = [
                np.concatenate([np.asarray(m[nm]) for m in in_maps], axis=0)
                for nm in in_names
            ]
            concat_zeros = [
                np.zeros((B * a.shape[0], *a.shape[1:]), a.dtype) for a in out_avals
            ]
            device_args = [jax.device_put(a) for a in concat_in + concat_zeros]
        out_arrs = sharded(*device_args)
        jax.block_until_ready(out_arrs)
        timings = []
        for _ in range(timing_reps):
            import time as _time

            t0 = _time.perf_counter()
            out_arrs = sharded(*device_args)
            jax.block_until_ready(out_arrs)
            timings.append(_time.perf_counter() - t0)
        results = [
            {
                nm: np.asarray(out_arrs[i]).reshape(B, *out_avals[i].shape)[c]
                for i, nm in enumerate(out_names)
            }
            for c in range(B)
        ]
        return results, device_args, timings

    _RUNNER_CACHE = run
    return run


def _pretile(w, nt, kt):
    """W [K, N] -> [nt, kt, 128, 128] with tile[n][k] = W[kblk, nblk]."""
    t = w.reshape(kt, P, nt, P).transpose(2, 0, 1, 3)
    return np.ascontiguousarray(t)


def _cols(vec):
    """[X*128] -> [128, X] with col j = vec[j*128:(j+1)*128]."""
    return np.ascontiguousarray(vec.reshape(-1, P).T)


def _build_in_maps(inputs):
    inp = {k: np.asarray(v, dtype=np.float32) for k, v in inputs.items()}
    x = inp["hidden_states"]  # [B, S, E]

    def _ptp(t):
        """[n, k, P, P] tile layout -> [n, P(p), k, P(m)] for contiguous DMA."""
        return np.ascontiguousarray(t.transpose(0, 2, 1, 3))

    wq = _ptp(_pretile(inp["Wq"] * WS, NE, NE)).astype(NPFP8)
    wk = _ptp(_pretile(inp["Wk"] * WS, NE, NE)).astype(NPFP8)
    wd = _ptp(_pretile(inp["Wd"] * WS, NE, NE)).astype(NPFP8)
    wi_t = _ptp(_pretile(inp["Wi"], NF, NE))  # [NF, P, NE, P]
    wi8 = np.ascontiguousarray(wi_t[:, :, 0:2, :] * WS).astype(NPFP8)
    wi = np.ascontiguousarray(wi_t[:, :, 2:, :] * (XS * WS)).astype(NPBF16)
    wo = np.ascontiguousarray(
        _pretile(inp["Wo"], NE, NF).reshape(NE, 4, 8, P, P).transpose(0, 1, 3, 2, 4)
    ).astype(NPBF16)
    # wv [NC, NE//2, P, 2, C]: wv[c, kp, p, i, j] = Wv[(2kp+i)P + p, cC + j]
    wv = (inp["Wv"] * WS).reshape(NE // 2, 2, P, NC, C)
    wv = np.ascontiguousarray(wv.transpose(3, 0, 2, 1, 4)).astype(NPFP8)

    zero = np.zeros_like(inp["bq"])
    bias_full = np.concatenate(
        [
            _cols(inp["bq"]), _cols(inp["bk"]),
            _cols(zero), _cols(zero),
            _cols(inp["g1"]), _cols(inp["b1"]),
            _cols(inp["bo"]), _cols(inp["g2"]), _cols(inp["b2"]),
        ],
        axis=1,
    )
    bicol = _cols(inp["bi"])
    c_fold = inp["bd"] + inp["bv"] @ inp["Wd"]  # LN1 residual bias

    in_maps = []
    for bidx in range(B):
        xT = np.ascontiguousarray(x[bidx].T)  # [E, S] f32
        xT8 = (xT * XS).astype(NPFP8)
        xTc = (xT + c_fold[:, None]).astype(NPBF16)
        in_maps.append(
            {
                "xT8": xT8, "xTc": xTc, "wq": wq, "wk": wk, "wv": wv,
                "wd": wd, "wi": wi, "wi8": wi8, "wo": wo, "biases": bias_full,
                "bi_cols": bicol,
                "ones_in": np.ones((P, 1), dtype=NPBF16),
                "ones8_in": np.ones((P, 1), dtype=NPFP8),
                "onesrow_in": np.ones((1, P), dtype=NPBF16),
            }
        )
    return in_maps


def kernel(**inputs):
    run = _get_runner()
    results, _, _ = run(_build_in_maps(inputs))
    out = np.stack([r["outT"].T for r in results]).astype(np.float32)
    return out
